# revision 32
# baseline (speedup 1.0000x reference)
"""LoRA linear on 8 trn2 NeuronCores.

out = x @ W.T + b + 2.0 * ((x @ A.T) @ B.T)
x [8192, 4096] f32, W [4096, 4096], b [4096], A [16, 4096], B [4096, 16].

Sharding: data-parallel over tokens (8 x 1024).

Main path runs in fp8 e4m3 with perf_mode=DoubleRow (2 fp8 weights per PE
cell -> 256-deep contraction per matmul, ~1.5x bf16 rate).  Inputs are
pre-scaled on host (x*32, W*1024, both well inside e4m3 range) and the
2^-15 compensation is folded into the final activation's scale.  The LoRA
path dominates the output's magnitude (std ~5 vs ~1.3 for the base term),
so it stays high precision: x/A in bf16 for x@A.T, and the rank-16 B-apply
+ bias accumulate into the same PSUM group in f32r.  Measured end-to-end
rel err ~8e-3 (gate 2e-2).
"""

import os
import sys
import types

for _p in ("/opt/trn_rl_repo", "/root/.axon_site/_ro/trn_rl_repo"):
    if os.path.isdir(_p) and _p not in sys.path:
        sys.path.append(_p)

import numpy as np
import ml_dtypes


def _ensure_axon_hooks():
    """bass_utils trace=True needs antenv.axon_hooks; some images lack it."""
    try:
        import antenv.axon_hooks  # noqa: F401
        return
    except Exception:
        pass
    mod = types.ModuleType("antenv.axon_hooks")
    mod._hook = None

    def set_axon_ntff_profile_hook(hook):
        mod._hook = hook

    def get_axon_ntff_profile_hook():
        if mod._hook is None:
            try:
                from trn_agent_boot.trn_boot import _ntff_profile_via_ctypes

                mod._hook = _ntff_profile_via_ctypes("/opt/axon/libaxon_pjrt.so")
            except Exception:
                return None
        return mod._hook

    mod.set_axon_ntff_profile_hook = set_axon_ntff_profile_hook
    mod.get_axon_ntff_profile_hook = get_axon_ntff_profile_hook
    try:
        import antenv

        antenv.axon_hooks = mod
    except Exception:
        pass
    sys.modules["antenv.axon_hooks"] = mod


_ensure_axon_hooks()

import concourse.bass as bass
import concourse.bass_utils as bass_utils
import concourse.mybir as mybir
import concourse.tile as tile_mod
from concourse.bass_utils import run_bass_kernel_spmd

# no fish bucket inside the container; keep artifacts local
bass_utils.upload_artifacts = lambda tmpdir: tmpdir


# ---------------------------------------------------------------------------
# Workarounds for this walrus build: it rejects any instruction that carries
# more than one semaphore wait ("Too many sync wait commands").  (a) replace
# the TileContext tail drain (stacks the whole global clock on one Drain),
# (b) split every multi-wait instruction in the serialized BIR into
# single-wait NoOps placed immediately before it (waits are AND conditions,
# so sequential single waits on the same engine are equivalent).
# ---------------------------------------------------------------------------


def _install_patches():
    from concourse.vector_clock import ScopedClock

    if not getattr(tile_mod.TileContext, "_drain_patch_installed", False):

        def _drain_and_barrier(self, tick_clock, wait_clock):
            nop_inst = self.nc.sync.nop(nofuse=True, hint="pre_drain_waits")
            wait_clock.add_sem_waits(
                nop_inst.ins, ScopedClock({None: tick_clock.global_clock})
            )
            si = nop_inst.ins.sync_info
            if si is not None and si.on_wait and len(si.on_wait) > 1:
                waits = list(si.on_wait)
                si.on_wait = waits[:1]
                for w in waits[1:]:
                    n2 = self.nc.sync.nop(nofuse=True, hint="pre_drain_waits")
                    n2.ins.sync_info = mybir.SyncInfo(on_wait=[w], on_update=[])
            self.nc.sync.drain()
            self.nc.all_engine_barrier()
            assert self.sems is not None
            popped = self.nc._tile_sem_poison_stack.pop()
            assert popped is self._sem_poison
            self.nc.clear_and_free_semaphores(list(self.sems.allocated().values()))
            self.nc.all_engine_barrier()

        tile_mod.TileContext._drain_and_barrier = _drain_and_barrier
        tile_mod.TileContext._drain_patch_installed = True

    if not getattr(bass.Bass, "_wait_split_installed", False):
        import json

        def _split_waits_json(raw):
            d = json.loads(raw)
            n = 0
            for f in d.get("functions", []):
                for b in f.get("blocks", []):
                    out = []
                    for inst in b.get("instructions", []):
                        si = inst.get("sync_info")
                        if si:
                            waits = si.get("on_wait") or []
                            if len(waits) > 1:
                                for w in waits[:-1]:
                                    n += 1
                                    nop = {
                                        "engine": inst["engine"],
                                        "ins": [],
                                        "outs": [],
                                        "name": f"wsplit-{n}",
                                        "opcode": "NoOp",
                                        "sync_info": {
                                            "on_update": [],
                                            "on_wait": [w],
                                        },
                                        "text_hint": "wsplit",
                                    }
                                    if "debug" in inst:
                                        nop["debug"] = inst["debug"]
                                    out.append(nop)
                                si["on_wait"] = [waits[-1]]
                        out.append(inst)
                    b["instructions"] = out
            return json.dumps(d).encode()

        def to_json_bytes(self):
            return _split_waits_json(mybir.module_to_json_bytes(self.m))

        bass.Bass.to_json_bytes = to_json_bytes
        bass.Bass._wait_split_installed = True


_install_patches()

# ---------------------------------------------------------------------------

N_CORES = 8
NTOK = 8192
K = 4096
O = 4096
R = 16
SCALING = 2.0

T = NTOK // N_CORES      # 1024 tokens per core
KC = K // 128            # 32 k-chunks of 128
KS = K // 256            # 16 k-superchunks of 256 (DoubleRow pairs)
OT = O // 128            # 32 o-tiles
TT = T // 512            # 2 token tiles of 512

SX = 32.0                # x fp8 pre-scale
SW = 1024.0              # W fp8 pre-scale
S = SX * SW              # 32768; PSUM holds S * (base + lora)

F32 = mybir.dt.float32
F32R = mybir.dt.float32r
BF16 = mybir.dt.bfloat16
F8 = mybir.dt.float8e4

LAST_RESULT = None  # test harness reads exec_time_ns off this


def _build_kernel():
    nc = bass.Bass("TRN2", num_devices=N_CORES)

    x8_in = nc.declare_dram_parameter("x8", [128, KC, T], F8, isOutput=False)
    xbf_in = nc.declare_dram_parameter("xbf", [128, KC, T], BF16, isOutput=False)
    w8_in = nc.declare_dram_parameter("w8", [OT, 128, KS, 2, 128], F8, isOutput=False)
    at_in = nc.declare_dram_parameter("at", [128, KC, R], BF16, isOutput=False)
    btb_in = nc.declare_dram_parameter("btb", [128, O], F32R, isOutput=False)
    btb2_in = nc.declare_dram_parameter("btb2", [128, O], F32R, isOutput=False)
    b_in = nc.declare_dram_parameter("b", [128, OT], F32, isOutput=False)
    y_out = nc.declare_dram_parameter("y", [OT, 128, T], BF16, isOutput=True)

    DR = mybir.MatmulPerfMode.DoubleRow

    with tile_mod.TileContext(nc) as tc:
        with (
            tc.tile_pool(name="xp", bufs=1) as xp,
            tc.tile_pool(name="cp", bufs=1) as cp,
            tc.tile_pool(name="wp", bufs=3) as wp,
            tc.tile_pool(name="op", bufs=2) as op,
            tc.tile_pool(name="dp", bufs=8) as dp,
            tc.tile_pool(name="psxa", bufs=2, space="PSUM") as psxa,
            tc.tile_pool(name="psp", bufs=6, space="PSUM") as psp,
        ):
            at_sb = cp.tile([128, KC, R], BF16)
            nc.scalar.dma_start(at_sb[:], at_in[:])
            b_sb = cp.tile([128, OT], F32)
            nc.scalar.dma_start(b_sb[:], b_in[:])
            # btb/btb2 (2 MiB each) are not needed until ~wave 4; their DMAs
            # are emitted at wave 2 so they don't delay x8/xbf/w8 here
            btb_sb = cp.tile([128, O], F32R)
            btb2_sb = cp.tile([128, O], F32R)

            # HAM warmup: the PE clock doubles only after ~3.4us of sustained
            # activity, and the first real matmuls wait ~10us for x8/w8 DMA.
            # Burn that window with throwaway fp32 matmuls (4 cyc/row) on a
            # zeroed scratch tile so the real stream starts at 2.4 GHz.
            warm_sb = cp.tile([128, 512], F32)
            nc.vector.memset(warm_sb[:], 0.0)
            for wi in range(6):
                pw = psp.tile([128, 512], F32, tag="pt", name=f"warm{wi}")
                nc.tensor.matmul(
                    pw[:], warm_sb[:, 0:128], warm_sb[:], start=True, stop=True
                )

            # x shard resident in SBUF, split in 8 so compute starts early.
            # x8 goes on the gpsimd queue by itself (the mains' only x
            # dependency); xbf follows on the scalar queue — with the lora
            # applies deferred for early waves, xa isn't needed until ~wave 4
            # so the slower xbf load is off the critical path.
            XG = 8
            GC = KC // XG  # 4 k-chunks per group
            XBG = 4
            GB = KC // XBG
            x8_parts = []
            xbf_parts = []
            for g in range(XG):
                xt = xp.tile([128, GC, T], F8, tag=f"x8{g}")
                nc.gpsimd.dma_start(xt[:], x8_in[:, g * GC:(g + 1) * GC, :])
                x8_parts.append(xt)
            for g in range(XBG):
                xt = xp.tile([128, GB, T], BF16, tag=f"xbf{g}")
                xbf_parts.append(xt)

            def emit_xbf_dmas():  # deferred to wave 1: x8 owns early HBM BW
                for g in range(XBG):
                    nc.scalar.dma_start(
                        xbf_parts[g][:], xbf_in[:, g * GB:(g + 1) * GB, :]
                    )

            def x8_sl(ks, t0):  # [128, 2, 512] fp8 rhs pair-chunk
                c = 2 * ks
                return x8_parts[c // GC][
                    :, c % GC:c % GC + 2, t0 * 512:(t0 + 1) * 512
                ]

            def xbf_sl(k, t0):  # [128, 512] bf16 rhs chunk
                return xbf_parts[k // GB][:, k % GB, t0 * 512:(t0 + 1) * 512]

            # LoRA xa = (x @ A.T).T in bf16; emitted after wave 0's mains so
            # the fp8 mains start as soon as x8 part 0 + W tiles land.
            # 4-way column tiling: strip j owns k-chunks {4m+j} and computes
            # into PSUM partitions 32j..32j+16 of one bank concurrently with
            # the other strips (separate XBUSes), ~4x faster than serial
            # M=16 matmuls.  The strips are then summed and replicated to all
            # four partition groups so the lora-apply can row-tile.
            xa_sb = cp.tile([128, T], F32R)
            ADD = mybir.AluOpType.add

            def emit_xa():
                for t in range(TT):
                    ts = slice(t * 512, (t + 1) * 512)
                    px = psxa.tile([128, 512], F32, tag="psxa", name=f"psxa{t}")
                    for m in range(KC // 4):
                        for j in range(4):
                            nc.tensor.matmul(
                                px[32 * j:32 * j + R, :],
                                at_sb[:, 4 * m + j, :],
                                xbf_sl(4 * m + j, t),
                                start=(m == 0),
                                stop=(m == KC // 4 - 1),
                                tile_position=(0, 32 * j),
                            )
                    # DVE may read at most one PSUM operand per instruction
                    nc.vector.tensor_copy(xa_sb[0:R, ts], px[0:R, :])
                    for j in range(1, 4):
                        nc.vector.tensor_tensor(
                            xa_sb[0:R, ts],
                            xa_sb[0:R, ts],
                            px[32 * j:32 * j + R, :],
                            ADD,
                        )
                    for j in range(1, 4):
                        nc.vector.tensor_copy(
                            xa_sb[32 * j:32 * j + R, ts], xa_sb[0:R, ts]
                        )

            # Early waves close their PSUM groups with base-only activations
            # (no xa dependency -> no pipeline stall while xbf streams in);
            # their lora term is patched in later from a separate rank-16
            # matmul (unscaled btb2) + DVE add, one o-tile per steady-state
            # wave.  Later waves fold the lora into the main PSUM group
            # (scaled btb).  The first two waves carry one o-tile each so
            # the very first matmul only needs w8[0] + 0.5 MiB of x8.
            WAVES = [[0], [1]] + [[i, i + 1] for i in range(2, OT, 2)]
            DEF_W = 5   # waves 0..4 = o-tiles 0..7 deferred
            N_FIX = 8
            def_tiles = {}

            def emit_fixup(ot):
                o_sb = def_tiles.pop(ot)
                for t in range(TT):
                    ts = slice(t * 512, (t + 1) * 512)
                    j = 2 * (ot % 2) + t
                    pf = psxa.tile([128, 512], F32, tag="psxa", name=f"fix{ot}_{t}")
                    nc.tensor.matmul(
                        pf[:],
                        btb2_sb[32 * j:32 * j + R, ot * 128:(ot + 1) * 128],
                        xa_sb[32 * j:32 * j + R, t * 512:(t + 1) * 512],
                        start=True,
                        stop=True,
                        tile_position=(32 * j, 0),
                    )
                    nc.vector.tensor_tensor(
                        o_sb[:, ts], o_sb[:, ts], pf[:], ADD
                    )
                    nc.gpsimd.dma_start(y_out[ot][:, ts], o_sb[:, ts])

            for wave, ots in enumerate(WAVES):
                w_tiles = []
                for ot in ots:
                    w_sb = wp.tile([128, KS, 2, 128], F8, tag="w", name=f"w{ot}")
                    nc.sync.dma_start(w_sb[:], w8_in[ot])
                    w_tiles.append(w_sb)
                pts = [
                    [
                        psp.tile([128, 512], F32, tag="pt", name=f"pt{ot}_{t}")
                        for t in range(TT)
                    ]
                    for ot in ots
                ]
                deferred = wave < DEF_W
                for ks in range(KS):
                    for otl in range(len(ots)):
                        for t in range(TT):
                            nc.tensor.matmul(
                                pts[otl][t][:],
                                w_tiles[otl][:, ks],
                                x8_sl(ks, t),
                                start=(ks == 0),
                                stop=(deferred and ks == KS - 1),
                                perf_mode=DR,
                            )
                if wave == 2:
                    emit_xbf_dmas()
                if wave == 3:
                    nc.sync.dma_start(btb_sb[:], btb_in[:])
                    nc.scalar.dma_start(btb2_sb[:], btb2_in[:])
                if wave == 4:
                    emit_xa()
                if deferred:
                    # close with a base-only epilogue; lora is patched later
                    o_sbs = [
                        dp.tile([128, T], BF16, tag="def", name=f"def{ot}")
                        for ot in ots
                    ]
                    for otl, ot in enumerate(ots):
                        def_tiles[ot] = o_sbs[otl]
                else:
                    # rank-16 lora applies, row-tiled: the 4 (ot, t) matmuls
                    # of this wave use disjoint 32-row strips of the PE array
                    # (the replicated xa/btb copies at partition 32j) so they
                    # run concurrently instead of costing 4x 512 cycles.
                    o_sbs = [
                        op.tile([128, T], BF16, tag="o", name=f"o{ot}")
                        for ot in ots
                    ]
                    for otl, ot in enumerate(ots):
                        for t in range(TT):
                            j = 2 * otl + t
                            nc.tensor.matmul(
                                pts[otl][t][:],
                                btb_sb[32 * j:32 * j + R, ot * 128:(ot + 1) * 128],
                                xa_sb[32 * j:32 * j + R, t * 512:(t + 1) * 512],
                                start=False,
                                stop=True,
                                tile_position=(32 * j, 0),
                            )
                for otl, ot in enumerate(ots):
                    for t in range(TT):
                        ts = slice(t * 512, (t + 1) * 512)
                        nc.scalar.activation(
                            o_sbs[otl][:, ts],
                            pts[otl][t][:],
                            mybir.ActivationFunctionType.Identity,
                            bias=b_sb[:, ot:ot + 1],
                            scale=1.0 / S,
                        )
                        if not deferred:
                            nc.gpsimd.dma_start(y_out[ot][:, ts], o_sbs[otl][:, ts])
                # patch one deferred o-tile per steady-state wave
                if DEF_W <= wave < DEF_W + N_FIX:
                    emit_fixup(wave - DEF_W)

    return nc


def kernel(x, W, b, A, B):
    global LAST_RESULT
    x = np.ascontiguousarray(x, dtype=np.float32)
    W = np.ascontiguousarray(W, dtype=np.float32)

    # host layout prep (transposes so the contraction dim lands on SBUF
    # partitions; blocked so every DMA is one fully-contiguous transfer)
    xT = x.T.reshape(KC, 128, N_CORES, T).transpose(2, 1, 0, 3)  # [core, p, kc, t]
    x8_dev = np.ascontiguousarray((xT * SX)).astype(ml_dtypes.float8_e4m3)
    xbf_dev = np.ascontiguousarray(xT).astype(ml_dtypes.bfloat16)
    # [ot, p, ks, i, m]: W.T[256*ks + 128*i + p, 128*ot + m] * SW
    w8_dev = np.ascontiguousarray(
        (W.T * SW).reshape(KS, 2, 128, OT, 128).transpose(3, 2, 0, 1, 4)
    ).astype(ml_dtypes.float8_e4m3)
    at_dev = np.ascontiguousarray(
        A.T.reshape(KC, 128, R).transpose(1, 0, 2)
    ).astype(ml_dtypes.bfloat16)  # [p, kc, r]
    # [128, O]: scaled B.T replicated at partition offsets 0/32/64/96 so the
    # lora-apply matmuls can be row-tiled to 4 concurrent 32-row strips.
    # btb folds the fp8 PSUM scale S (inline applies); btb2 is unscaled (the
    # deferred fixups add into already-descaled bf16 output tiles).
    btb_dev = np.zeros((128, O), dtype=np.float32)
    btb2_dev = np.zeros((128, O), dtype=np.float32)
    for j in range(4):
        btb_dev[32 * j:32 * j + R] = S * SCALING * B.T.astype(np.float32)
        btb2_dev[32 * j:32 * j + R] = SCALING * B.T.astype(np.float32)
    b_dev = np.ascontiguousarray(
        np.asarray(b, dtype=np.float32).reshape(OT, 128).T
    )  # [p, ot]

    nc = _build_kernel()
    in_maps = [
        {
            "x8": x8_dev[c],
            "xbf": xbf_dev[c],
            "w8": w8_dev,
            "at": at_dev,
            "btb": btb_dev,
            "btb2": btb2_dev,
            "b": b_dev,
        }
        for c in range(N_CORES)
    ]
    res = run_bass_kernel_spmd(nc, in_maps, list(range(N_CORES)))
    LAST_RESULT = res

    out = np.stack(
        [res.results[c]["y"].astype(np.float32) for c in range(N_CORES)]
    )  # [c, ot, o, t]
    return np.ascontiguousarray(
        out.transpose(0, 3, 1, 2).reshape(NTOK, O)
    )


# revision 36
# speedup vs baseline: 1.0039x; 1.0039x over previous
"""LoRA linear on 8 trn2 NeuronCores.

out = x @ W.T + b + 2.0 * ((x @ A.T) @ B.T)
x [8192, 4096] f32, W [4096, 4096], b [4096], A [16, 4096], B [4096, 16].

Sharding: data-parallel over tokens (8 x 1024).

Main path runs in fp8 e4m3 with perf_mode=DoubleRow (2 fp8 weights per PE
cell -> 256-deep contraction per matmul, ~1.5x bf16 rate).  Inputs are
pre-scaled on host (x*32, W*1024, both well inside e4m3 range) and the
2^-15 compensation is folded into the final activation's scale.  The LoRA
path dominates the output's magnitude (std ~5 vs ~1.3 for the base term),
so it stays high precision: x/A in bf16 for x@A.T, and the rank-16 B-apply
+ bias accumulate into the same PSUM group in f32r.  Measured end-to-end
rel err ~8e-3 (gate 2e-2).
"""

import os
import sys
import types

for _p in ("/opt/trn_rl_repo", "/root/.axon_site/_ro/trn_rl_repo"):
    if os.path.isdir(_p) and _p not in sys.path:
        sys.path.append(_p)

import numpy as np
import ml_dtypes


def _ensure_axon_hooks():
    """bass_utils trace=True needs antenv.axon_hooks; some images lack it."""
    try:
        import antenv.axon_hooks  # noqa: F401
        return
    except Exception:
        pass
    mod = types.ModuleType("antenv.axon_hooks")
    mod._hook = None

    def set_axon_ntff_profile_hook(hook):
        mod._hook = hook

    def get_axon_ntff_profile_hook():
        if mod._hook is None:
            try:
                from trn_agent_boot.trn_boot import _ntff_profile_via_ctypes

                mod._hook = _ntff_profile_via_ctypes("/opt/axon/libaxon_pjrt.so")
            except Exception:
                return None
        return mod._hook

    mod.set_axon_ntff_profile_hook = set_axon_ntff_profile_hook
    mod.get_axon_ntff_profile_hook = get_axon_ntff_profile_hook
    try:
        import antenv

        antenv.axon_hooks = mod
    except Exception:
        pass
    sys.modules["antenv.axon_hooks"] = mod


_ensure_axon_hooks()

import concourse.bass as bass
import concourse.bass_utils as bass_utils
import concourse.mybir as mybir
import concourse.tile as tile_mod
from concourse.bass_utils import run_bass_kernel_spmd

# no fish bucket inside the container; keep artifacts local
bass_utils.upload_artifacts = lambda tmpdir: tmpdir


# ---------------------------------------------------------------------------
# Workarounds for this walrus build: it rejects any instruction that carries
# more than one semaphore wait ("Too many sync wait commands").  (a) replace
# the TileContext tail drain (stacks the whole global clock on one Drain),
# (b) split every multi-wait instruction in the serialized BIR into
# single-wait NoOps placed immediately before it (waits are AND conditions,
# so sequential single waits on the same engine are equivalent).
# ---------------------------------------------------------------------------


def _install_patches():
    from concourse.vector_clock import ScopedClock

    if not getattr(tile_mod.TileContext, "_drain_patch_installed", False):

        def _drain_and_barrier(self, tick_clock, wait_clock):
            nop_inst = self.nc.sync.nop(nofuse=True, hint="pre_drain_waits")
            wait_clock.add_sem_waits(
                nop_inst.ins, ScopedClock({None: tick_clock.global_clock})
            )
            si = nop_inst.ins.sync_info
            if si is not None and si.on_wait and len(si.on_wait) > 1:
                waits = list(si.on_wait)
                si.on_wait = waits[:1]
                for w in waits[1:]:
                    n2 = self.nc.sync.nop(nofuse=True, hint="pre_drain_waits")
                    n2.ins.sync_info = mybir.SyncInfo(on_wait=[w], on_update=[])
            self.nc.sync.drain()
            self.nc.all_engine_barrier()
            assert self.sems is not None
            popped = self.nc._tile_sem_poison_stack.pop()
            assert popped is self._sem_poison
            self.nc.clear_and_free_semaphores(list(self.sems.allocated().values()))
            self.nc.all_engine_barrier()

        tile_mod.TileContext._drain_and_barrier = _drain_and_barrier
        tile_mod.TileContext._drain_patch_installed = True

    if not getattr(bass.Bass, "_wait_split_installed", False):
        import json

        def _split_waits_json(raw):
            d = json.loads(raw)
            n = 0
            for f in d.get("functions", []):
                for b in f.get("blocks", []):
                    out = []
                    for inst in b.get("instructions", []):
                        si = inst.get("sync_info")
                        if si:
                            waits = si.get("on_wait") or []
                            if len(waits) > 1:
                                for w in waits[:-1]:
                                    n += 1
                                    nop = {
                                        "engine": inst["engine"],
                                        "ins": [],
                                        "outs": [],
                                        "name": f"wsplit-{n}",
                                        "opcode": "NoOp",
                                        "sync_info": {
                                            "on_update": [],
                                            "on_wait": [w],
                                        },
                                        "text_hint": "wsplit",
                                    }
                                    if "debug" in inst:
                                        nop["debug"] = inst["debug"]
                                    out.append(nop)
                                si["on_wait"] = [waits[-1]]
                        out.append(inst)
                    b["instructions"] = out
            return json.dumps(d).encode()

        def to_json_bytes(self):
            return _split_waits_json(mybir.module_to_json_bytes(self.m))

        bass.Bass.to_json_bytes = to_json_bytes
        bass.Bass._wait_split_installed = True


_install_patches()

# ---------------------------------------------------------------------------

N_CORES = 8
NTOK = 8192
K = 4096
O = 4096
R = 16
SCALING = 2.0

T = NTOK // N_CORES      # 1024 tokens per core
KC = K // 128            # 32 k-chunks of 128
KS = K // 256            # 16 k-superchunks of 256 (DoubleRow pairs)
OT = O // 128            # 32 o-tiles
TT = T // 512            # 2 token tiles of 512

SX = 32.0                # x fp8 pre-scale
SW = 1024.0              # W fp8 pre-scale
S = SX * SW              # 32768; PSUM holds S * (base + lora)

F32 = mybir.dt.float32
F32R = mybir.dt.float32r
BF16 = mybir.dt.bfloat16
F8 = mybir.dt.float8e4

LAST_RESULT = None  # test harness reads exec_time_ns off this


def _build_kernel():
    nc = bass.Bass("TRN2", num_devices=N_CORES)

    x8_in = nc.declare_dram_parameter("x8", [128, KC, T], F8, isOutput=False)
    xbf_in = nc.declare_dram_parameter("xbf", [128, KC, T], BF16, isOutput=False)
    w8_in = nc.declare_dram_parameter("w8", [OT, 128, KS, 2, 128], F8, isOutput=False)
    at_in = nc.declare_dram_parameter("at", [128, KC, R], BF16, isOutput=False)
    btb_in = nc.declare_dram_parameter("btb", [128, O], F32R, isOutput=False)
    btb2_in = nc.declare_dram_parameter("btb2", [128, O], F32R, isOutput=False)
    b_in = nc.declare_dram_parameter("b", [128, OT], F32, isOutput=False)
    y_out = nc.declare_dram_parameter("y", [OT, 128, T], BF16, isOutput=True)

    DR = mybir.MatmulPerfMode.DoubleRow

    with tile_mod.TileContext(nc) as tc:
        with (
            tc.tile_pool(name="xp", bufs=1) as xp,
            tc.tile_pool(name="cp", bufs=1) as cp,
            tc.tile_pool(name="wp", bufs=3) as wp,
            tc.tile_pool(name="op", bufs=2) as op,
            tc.tile_pool(name="dp", bufs=8) as dp,
            tc.tile_pool(name="psxa", bufs=2, space="PSUM") as psxa,
            tc.tile_pool(name="psp", bufs=6, space="PSUM") as psp,
        ):
            at_sb = cp.tile([128, KC, R], BF16)
            nc.scalar.dma_start(at_sb[:], at_in[:])
            b_sb = cp.tile([128, OT], F32)
            nc.scalar.dma_start(b_sb[:], b_in[:])
            # btb/btb2 (2 MiB each) are not needed until ~wave 4; their DMAs
            # are emitted at wave 2 so they don't delay x8/xbf/w8 here
            btb_sb = cp.tile([128, O], F32R)
            btb2_sb = cp.tile([128, O], F32R)

            # HAM warmup: the PE clock doubles only after ~3.4us of sustained
            # activity, and the first real matmuls wait ~10us for x8/w8 DMA.
            # Burn that window with throwaway fp32 matmuls (4 cyc/row) on a
            # zeroed scratch tile so the real stream starts at 2.4 GHz.
            warm_sb = cp.tile([128, 512], F32)
            nc.vector.memset(warm_sb[:], 0.0)
            for wi in range(6):
                pw = psp.tile([128, 512], F32, tag="pt", name=f"warm{wi}")
                nc.tensor.matmul(
                    pw[:], warm_sb[:, 0:128], warm_sb[:], start=True, stop=True
                )

            # x shard resident in SBUF, split in 8 so compute starts early.
            # x8 goes on the gpsimd queue by itself (the mains' only x
            # dependency); xbf follows on the scalar queue — with the lora
            # applies deferred for early waves, xa isn't needed until ~wave 4
            # so the slower xbf load is off the critical path.
            XG = 4
            GC = KC // XG  # 8 k-chunks per group
            XBG = 4
            GB = KC // XBG
            x8_parts = []
            xbf_parts = []
            for g in range(XG):
                xt = xp.tile([128, GC, T], F8, tag=f"x8{g}")
                nc.gpsimd.dma_start(xt[:], x8_in[:, g * GC:(g + 1) * GC, :])
                x8_parts.append(xt)
            for g in range(XBG):
                xt = xp.tile([128, GB, T], BF16, tag=f"xbf{g}")
                xbf_parts.append(xt)

            def emit_xbf_dmas():  # deferred to wave 1: x8 owns early HBM BW
                for g in range(XBG):
                    nc.scalar.dma_start(
                        xbf_parts[g][:], xbf_in[:, g * GB:(g + 1) * GB, :]
                    )

            def x8_sl(ks, t0):  # [128, 2, 512] fp8 rhs pair-chunk
                c = 2 * ks
                return x8_parts[c // GC][
                    :, c % GC:c % GC + 2, t0 * 512:(t0 + 1) * 512
                ]

            def xbf_sl(k, t0):  # [128, 512] bf16 rhs chunk
                return xbf_parts[k // GB][:, k % GB, t0 * 512:(t0 + 1) * 512]

            # LoRA xa = (x @ A.T).T in bf16; emitted after wave 0's mains so
            # the fp8 mains start as soon as x8 part 0 + W tiles land.
            # 4-way column tiling: strip j owns k-chunks {4m+j} and computes
            # into PSUM partitions 32j..32j+16 of one bank concurrently with
            # the other strips (separate XBUSes), ~4x faster than serial
            # M=16 matmuls.  The strips are then summed and replicated to all
            # four partition groups so the lora-apply can row-tile.
            xa_sb = cp.tile([128, T], F32R)
            ADD = mybir.AluOpType.add

            def emit_xa():
                for t in range(TT):
                    ts = slice(t * 512, (t + 1) * 512)
                    px = psxa.tile([128, 512], F32, tag="psxa", name=f"psxa{t}")
                    for m in range(KC // 4):
                        for j in range(4):
                            nc.tensor.matmul(
                                px[32 * j:32 * j + R, :],
                                at_sb[:, 4 * m + j, :],
                                xbf_sl(4 * m + j, t),
                                start=(m == 0),
                                stop=(m == KC // 4 - 1),
                                tile_position=(0, 32 * j),
                            )
                    # DVE may read at most one PSUM operand per instruction
                    nc.vector.tensor_copy(xa_sb[0:R, ts], px[0:R, :])
                    for j in range(1, 4):
                        nc.vector.tensor_tensor(
                            xa_sb[0:R, ts],
                            xa_sb[0:R, ts],
                            px[32 * j:32 * j + R, :],
                            ADD,
                        )
                    for j in range(1, 4):
                        nc.vector.tensor_copy(
                            xa_sb[32 * j:32 * j + R, ts], xa_sb[0:R, ts]
                        )

            # Early waves close their PSUM groups with base-only activations
            # (no xa dependency -> no pipeline stall while xbf streams in);
            # their lora term is patched in later from a separate rank-16
            # matmul (unscaled btb2) + DVE add, one o-tile per steady-state
            # wave.  Later waves fold the lora into the main PSUM group
            # (scaled btb).  The first wave carries THREE o-tiles whose ks
            # loops are interleaved by x8 part: while the x shard trickles in
            # at the DMA ramp rate, each arriving MiB unlocks 3x32 matmuls
            # instead of 16, keeping the PE backlog deep enough to ride out
            # bursty arrivals (the 3 o-tiles use all 6 PSUM mains banks).
            WAVES = [[0, 1, 2], [3]] + [[i, i + 1] for i in range(4, OT, 2)]
            DEF_W = 4   # waves 0..3 = o-tiles 0..7 deferred
            N_FIX = 8
            FIX_W0 = 5  # first fixup wave
            def_tiles = {}

            def emit_fixup(ot):
                o_sb = def_tiles.pop(ot)
                for t in range(TT):
                    ts = slice(t * 512, (t + 1) * 512)
                    j = 2 * (ot % 2) + t
                    pf = psxa.tile([128, 512], F32, tag="psxa", name=f"fix{ot}_{t}")
                    nc.tensor.matmul(
                        pf[:],
                        btb2_sb[32 * j:32 * j + R, ot * 128:(ot + 1) * 128],
                        xa_sb[32 * j:32 * j + R, t * 512:(t + 1) * 512],
                        start=True,
                        stop=True,
                        tile_position=(32 * j, 0),
                    )
                    nc.vector.tensor_tensor(
                        o_sb[:, ts], o_sb[:, ts], pf[:], ADD
                    )
                    nc.gpsimd.dma_start(y_out[ot][:, ts], o_sb[:, ts])

            for wave, ots in enumerate(WAVES):
                w_tiles = []
                for ot in ots:
                    w_sb = wp.tile([128, KS, 2, 128], F8, tag="w", name=f"w{ot}")
                    nc.sync.dma_start(w_sb[:], w8_in[ot])
                    w_tiles.append(w_sb)
                pts = [
                    [
                        psp.tile([128, 512], F32, tag="pt", name=f"pt{ot}_{t}")
                        for t in range(TT)
                    ]
                    for ot in ots
                ]
                deferred = wave < DEF_W
                if wave == 0:
                    # interleave the 3 o-tiles' ks loops by x8 part
                    for c in range(XG):
                        for otl in range(len(ots)):
                            for ks in range(4 * c, 4 * c + 4):
                                for t in range(TT):
                                    nc.tensor.matmul(
                                        pts[otl][t][:],
                                        w_tiles[otl][:, ks],
                                        x8_sl(ks, t),
                                        start=(ks == 0),
                                        stop=(deferred and ks == KS - 1),
                                        perf_mode=DR,
                                    )
                else:
                    for ks in range(KS):
                        for otl in range(len(ots)):
                            for t in range(TT):
                                nc.tensor.matmul(
                                    pts[otl][t][:],
                                    w_tiles[otl][:, ks],
                                    x8_sl(ks, t),
                                    start=(ks == 0),
                                    stop=(deferred and ks == KS - 1),
                                    perf_mode=DR,
                                )
                if wave == 1:
                    emit_xbf_dmas()
                if wave == 2:
                    nc.sync.dma_start(btb_sb[:], btb_in[:])
                    nc.scalar.dma_start(btb2_sb[:], btb2_in[:])
                if wave == 4:
                    emit_xa()
                if deferred:
                    # close with a base-only epilogue; lora is patched later
                    o_sbs = [
                        dp.tile([128, T], BF16, tag="def", name=f"def{ot}")
                        for ot in ots
                    ]
                    for otl, ot in enumerate(ots):
                        def_tiles[ot] = o_sbs[otl]
                else:
                    # rank-16 lora applies, row-tiled: the 4 (ot, t) matmuls
                    # of this wave use disjoint 32-row strips of the PE array
                    # (the replicated xa/btb copies at partition 32j) so they
                    # run concurrently instead of costing 4x 512 cycles.
                    o_sbs = [
                        op.tile([128, T], BF16, tag="o", name=f"o{ot}")
                        for ot in ots
                    ]
                    for otl, ot in enumerate(ots):
                        for t in range(TT):
                            j = 2 * otl + t
                            nc.tensor.matmul(
                                pts[otl][t][:],
                                btb_sb[32 * j:32 * j + R, ot * 128:(ot + 1) * 128],
                                xa_sb[32 * j:32 * j + R, t * 512:(t + 1) * 512],
                                start=False,
                                stop=True,
                                tile_position=(32 * j, 0),
                            )
                for otl, ot in enumerate(ots):
                    for t in range(TT):
                        ts = slice(t * 512, (t + 1) * 512)
                        nc.scalar.activation(
                            o_sbs[otl][:, ts],
                            pts[otl][t][:],
                            mybir.ActivationFunctionType.Identity,
                            bias=b_sb[:, ot:ot + 1],
                            scale=1.0 / S,
                        )
                        if not deferred:
                            nc.gpsimd.dma_start(y_out[ot][:, ts], o_sbs[otl][:, ts])
                # patch one deferred o-tile per steady-state wave
                if FIX_W0 <= wave < FIX_W0 + N_FIX:
                    emit_fixup(wave - FIX_W0)

    return nc


def kernel(x, W, b, A, B):
    global LAST_RESULT
    x = np.ascontiguousarray(x, dtype=np.float32)
    W = np.ascontiguousarray(W, dtype=np.float32)

    # host layout prep (transposes so the contraction dim lands on SBUF
    # partitions; blocked so every DMA is one fully-contiguous transfer)
    xT = x.T.reshape(KC, 128, N_CORES, T).transpose(2, 1, 0, 3)  # [core, p, kc, t]
    x8_dev = np.ascontiguousarray((xT * SX)).astype(ml_dtypes.float8_e4m3)
    xbf_dev = np.ascontiguousarray(xT).astype(ml_dtypes.bfloat16)
    # [ot, p, ks, i, m]: W.T[256*ks + 128*i + p, 128*ot + m] * SW
    w8_dev = np.ascontiguousarray(
        (W.T * SW).reshape(KS, 2, 128, OT, 128).transpose(3, 2, 0, 1, 4)
    ).astype(ml_dtypes.float8_e4m3)
    at_dev = np.ascontiguousarray(
        A.T.reshape(KC, 128, R).transpose(1, 0, 2)
    ).astype(ml_dtypes.bfloat16)  # [p, kc, r]
    # [128, O]: scaled B.T replicated at partition offsets 0/32/64/96 so the
    # lora-apply matmuls can be row-tiled to 4 concurrent 32-row strips.
    # btb folds the fp8 PSUM scale S (inline applies); btb2 is unscaled (the
    # deferred fixups add into already-descaled bf16 output tiles).
    btb_dev = np.zeros((128, O), dtype=np.float32)
    btb2_dev = np.zeros((128, O), dtype=np.float32)
    for j in range(4):
        btb_dev[32 * j:32 * j + R] = S * SCALING * B.T.astype(np.float32)
        btb2_dev[32 * j:32 * j + R] = SCALING * B.T.astype(np.float32)
    b_dev = np.ascontiguousarray(
        np.asarray(b, dtype=np.float32).reshape(OT, 128).T
    )  # [p, ot]

    nc = _build_kernel()
    in_maps = [
        {
            "x8": x8_dev[c],
            "xbf": xbf_dev[c],
            "w8": w8_dev,
            "at": at_dev,
            "btb": btb_dev,
            "btb2": btb2_dev,
            "b": b_dev,
        }
        for c in range(N_CORES)
    ]
    res = run_bass_kernel_spmd(nc, in_maps, list(range(N_CORES)))
    LAST_RESULT = res

    out = np.stack(
        [res.results[c]["y"].astype(np.float32) for c in range(N_CORES)]
    )  # [c, ot, o, t]
    return np.ascontiguousarray(
        out.transpose(0, 3, 1, 2).reshape(NTOK, O)
    )


# revision 37
# speedup vs baseline: 1.0127x; 1.0088x over previous
"""LoRA linear on 8 trn2 NeuronCores.

out = x @ W.T + b + 2.0 * ((x @ A.T) @ B.T)
x [8192, 4096] f32, W [4096, 4096], b [4096], A [16, 4096], B [4096, 16].

Sharding: data-parallel over tokens (8 x 1024).

Main path runs in fp8 e4m3 with perf_mode=DoubleRow (2 fp8 weights per PE
cell -> 256-deep contraction per matmul, ~1.5x bf16 rate).  Inputs are
pre-scaled on host (x*32, W*1024, both well inside e4m3 range) and the
2^-15 compensation is folded into the final activation's scale.  The LoRA
path dominates the output's magnitude (std ~5 vs ~1.3 for the base term),
so it stays high precision: x/A in bf16 for x@A.T, and the rank-16 B-apply
+ bias accumulate into the same PSUM group in f32r.  Measured end-to-end
rel err ~8e-3 (gate 2e-2).
"""

import os
import sys
import types

for _p in ("/opt/trn_rl_repo", "/root/.axon_site/_ro/trn_rl_repo"):
    if os.path.isdir(_p) and _p not in sys.path:
        sys.path.append(_p)

import numpy as np
import ml_dtypes


def _ensure_axon_hooks():
    """bass_utils trace=True needs antenv.axon_hooks; some images lack it."""
    try:
        import antenv.axon_hooks  # noqa: F401
        return
    except Exception:
        pass
    mod = types.ModuleType("antenv.axon_hooks")
    mod._hook = None

    def set_axon_ntff_profile_hook(hook):
        mod._hook = hook

    def get_axon_ntff_profile_hook():
        if mod._hook is None:
            try:
                from trn_agent_boot.trn_boot import _ntff_profile_via_ctypes

                mod._hook = _ntff_profile_via_ctypes("/opt/axon/libaxon_pjrt.so")
            except Exception:
                return None
        return mod._hook

    mod.set_axon_ntff_profile_hook = set_axon_ntff_profile_hook
    mod.get_axon_ntff_profile_hook = get_axon_ntff_profile_hook
    try:
        import antenv

        antenv.axon_hooks = mod
    except Exception:
        pass
    sys.modules["antenv.axon_hooks"] = mod


_ensure_axon_hooks()

import concourse.bass as bass
import concourse.bass_utils as bass_utils
import concourse.mybir as mybir
import concourse.tile as tile_mod
from concourse.bass_utils import run_bass_kernel_spmd

# no fish bucket inside the container; keep artifacts local
bass_utils.upload_artifacts = lambda tmpdir: tmpdir


# ---------------------------------------------------------------------------
# Workarounds for this walrus build: it rejects any instruction that carries
# more than one semaphore wait ("Too many sync wait commands").  (a) replace
# the TileContext tail drain (stacks the whole global clock on one Drain),
# (b) split every multi-wait instruction in the serialized BIR into
# single-wait NoOps placed immediately before it (waits are AND conditions,
# so sequential single waits on the same engine are equivalent).
# ---------------------------------------------------------------------------


def _install_patches():
    from concourse.vector_clock import ScopedClock

    if not getattr(tile_mod.TileContext, "_drain_patch_installed", False):

        def _drain_and_barrier(self, tick_clock, wait_clock):
            nop_inst = self.nc.sync.nop(nofuse=True, hint="pre_drain_waits")
            wait_clock.add_sem_waits(
                nop_inst.ins, ScopedClock({None: tick_clock.global_clock})
            )
            si = nop_inst.ins.sync_info
            if si is not None and si.on_wait and len(si.on_wait) > 1:
                waits = list(si.on_wait)
                si.on_wait = waits[:1]
                for w in waits[1:]:
                    n2 = self.nc.sync.nop(nofuse=True, hint="pre_drain_waits")
                    n2.ins.sync_info = mybir.SyncInfo(on_wait=[w], on_update=[])
            self.nc.sync.drain()
            self.nc.all_engine_barrier()
            assert self.sems is not None
            popped = self.nc._tile_sem_poison_stack.pop()
            assert popped is self._sem_poison
            self.nc.clear_and_free_semaphores(list(self.sems.allocated().values()))
            self.nc.all_engine_barrier()

        tile_mod.TileContext._drain_and_barrier = _drain_and_barrier
        tile_mod.TileContext._drain_patch_installed = True

    if not getattr(bass.Bass, "_wait_split_installed", False):
        import json

        def _split_waits_json(raw):
            d = json.loads(raw)
            n = 0
            for f in d.get("functions", []):
                for b in f.get("blocks", []):
                    out = []
                    for inst in b.get("instructions", []):
                        si = inst.get("sync_info")
                        if si:
                            waits = si.get("on_wait") or []
                            if len(waits) > 1:
                                for w in waits[:-1]:
                                    n += 1
                                    nop = {
                                        "engine": inst["engine"],
                                        "ins": [],
                                        "outs": [],
                                        "name": f"wsplit-{n}",
                                        "opcode": "NoOp",
                                        "sync_info": {
                                            "on_update": [],
                                            "on_wait": [w],
                                        },
                                        "text_hint": "wsplit",
                                    }
                                    if "debug" in inst:
                                        nop["debug"] = inst["debug"]
                                    out.append(nop)
                                si["on_wait"] = [waits[-1]]
                        out.append(inst)
                    b["instructions"] = out
            return json.dumps(d).encode()

        def to_json_bytes(self):
            return _split_waits_json(mybir.module_to_json_bytes(self.m))

        bass.Bass.to_json_bytes = to_json_bytes
        bass.Bass._wait_split_installed = True


_install_patches()

# ---------------------------------------------------------------------------

N_CORES = 8
NTOK = 8192
K = 4096
O = 4096
R = 16
SCALING = 2.0

T = NTOK // N_CORES      # 1024 tokens per core
KC = K // 128            # 32 k-chunks of 128
KS = K // 256            # 16 k-superchunks of 256 (DoubleRow pairs)
OT = O // 128            # 32 o-tiles
TT = T // 512            # 2 token tiles of 512

SX = 32.0                # x fp8 pre-scale
SW = 1024.0              # W fp8 pre-scale
S = SX * SW              # 32768; PSUM holds S * (base + lora)

F32 = mybir.dt.float32
F32R = mybir.dt.float32r
BF16 = mybir.dt.bfloat16
F8 = mybir.dt.float8e4

LAST_RESULT = None  # test harness reads exec_time_ns off this


def _build_kernel():
    nc = bass.Bass("TRN2", num_devices=N_CORES)

    x8_in = nc.declare_dram_parameter("x8", [128, KC, T], F8, isOutput=False)
    xbf_in = nc.declare_dram_parameter("xbf", [128, KC, T], BF16, isOutput=False)
    w8_in = nc.declare_dram_parameter("w8", [OT, 128, KS, 2, 128], F8, isOutput=False)
    at_in = nc.declare_dram_parameter("at", [128, KC, R], BF16, isOutput=False)
    btb_in = nc.declare_dram_parameter("btb", [128, O], F32R, isOutput=False)
    btb2_in = nc.declare_dram_parameter("btb2", [128, O], F32R, isOutput=False)
    b_in = nc.declare_dram_parameter("b", [128, OT], F32, isOutput=False)
    y_out = nc.declare_dram_parameter("y", [OT, 128, T], BF16, isOutput=True)

    DR = mybir.MatmulPerfMode.DoubleRow

    with tile_mod.TileContext(nc) as tc:
        with (
            tc.tile_pool(name="xp", bufs=1) as xp,
            tc.tile_pool(name="cp", bufs=1) as cp,
            tc.tile_pool(name="wp", bufs=3) as wp,
            tc.tile_pool(name="op", bufs=2) as op,
            tc.tile_pool(name="dp", bufs=8) as dp,
            tc.tile_pool(name="psxa", bufs=2, space="PSUM") as psxa,
            tc.tile_pool(name="psp", bufs=6, space="PSUM") as psp,
        ):
            at_sb = cp.tile([128, KC, R], BF16)
            nc.scalar.dma_start(at_sb[:], at_in[:])
            b_sb = cp.tile([128, OT], F32)
            nc.scalar.dma_start(b_sb[:], b_in[:])
            # btb/btb2 (2 MiB each) are not needed until ~wave 4; their DMAs
            # are emitted at wave 2 so they don't delay x8/xbf/w8 here
            btb_sb = cp.tile([128, O], F32R)
            btb2_sb = cp.tile([128, O], F32R)

            # HAM warmup: the PE clock doubles only after ~3.4us of sustained
            # activity, and the head is DMA-bound until ~24us (x8 + first W
            # tiles trickle in at the DMA ramp rate).  Pad that entire window
            # with throwaway fp32 matmuls (4 cyc/row) on a zeroed scratch
            # tile: the PE stays HAM-warm with nothing to do anyway, and the
            # real stream starts at 2.4 GHz right as the data lands.
            warm_sb = cp.tile([128, 512], F32)
            nc.vector.memset(warm_sb[:], 0.0)
            for wi in range(16):
                pw = psp.tile([128, 512], F32, tag="pt", name=f"warm{wi}")
                nc.tensor.matmul(
                    pw[:], warm_sb[:, 0:128], warm_sb[:], start=True, stop=True
                )

            # x shard resident in SBUF, split in 8 so compute starts early.
            # x8 goes on the gpsimd queue by itself (the mains' only x
            # dependency); xbf follows on the scalar queue — with the lora
            # applies deferred for early waves, xa isn't needed until ~wave 4
            # so the slower xbf load is off the critical path.
            XG = 4
            GC = KC // XG  # 8 k-chunks per group
            XBG = 4
            GB = KC // XBG
            x8_parts = []
            xbf_parts = []
            for g in range(XG):
                xt = xp.tile([128, GC, T], F8, tag=f"x8{g}")
                nc.gpsimd.dma_start(xt[:], x8_in[:, g * GC:(g + 1) * GC, :])
                x8_parts.append(xt)
            for g in range(XBG):
                xt = xp.tile([128, GB, T], BF16, tag=f"xbf{g}")
                xbf_parts.append(xt)

            def emit_xbf_dmas():  # deferred to wave 1: x8 owns early HBM BW
                for g in range(XBG):
                    nc.scalar.dma_start(
                        xbf_parts[g][:], xbf_in[:, g * GB:(g + 1) * GB, :]
                    )

            def x8_sl(ks, t0):  # [128, 2, 512] fp8 rhs pair-chunk
                c = 2 * ks
                return x8_parts[c // GC][
                    :, c % GC:c % GC + 2, t0 * 512:(t0 + 1) * 512
                ]

            def xbf_sl(k, t0):  # [128, 512] bf16 rhs chunk
                return xbf_parts[k // GB][:, k % GB, t0 * 512:(t0 + 1) * 512]

            # LoRA xa = (x @ A.T).T in bf16; emitted after wave 0's mains so
            # the fp8 mains start as soon as x8 part 0 + W tiles land.
            # 4-way column tiling: strip j owns k-chunks {4m+j} and computes
            # into PSUM partitions 32j..32j+16 of one bank concurrently with
            # the other strips (separate XBUSes), ~4x faster than serial
            # M=16 matmuls.  The strips are then summed and replicated to all
            # four partition groups so the lora-apply can row-tile.
            xa_sb = cp.tile([128, T], F32R)
            ADD = mybir.AluOpType.add

            def emit_xa():
                for t in range(TT):
                    ts = slice(t * 512, (t + 1) * 512)
                    px = psxa.tile([128, 512], F32, tag="psxa", name=f"psxa{t}")
                    for m in range(KC // 4):
                        for j in range(4):
                            nc.tensor.matmul(
                                px[32 * j:32 * j + R, :],
                                at_sb[:, 4 * m + j, :],
                                xbf_sl(4 * m + j, t),
                                start=(m == 0),
                                stop=(m == KC // 4 - 1),
                                tile_position=(0, 32 * j),
                            )
                    # DVE may read at most one PSUM operand per instruction
                    nc.vector.tensor_copy(xa_sb[0:R, ts], px[0:R, :])
                    for j in range(1, 4):
                        nc.vector.tensor_tensor(
                            xa_sb[0:R, ts],
                            xa_sb[0:R, ts],
                            px[32 * j:32 * j + R, :],
                            ADD,
                        )
                    for j in range(1, 4):
                        nc.vector.tensor_copy(
                            xa_sb[32 * j:32 * j + R, ts], xa_sb[0:R, ts]
                        )

            # Early waves close their PSUM groups with base-only activations
            # (no xa dependency -> no pipeline stall while xbf streams in);
            # their lora term is patched in later from a separate rank-16
            # matmul (unscaled btb2) + DVE add, one o-tile per steady-state
            # wave.  Later waves fold the lora into the main PSUM group
            # (scaled btb).  The first wave carries THREE o-tiles whose ks
            # loops are interleaved by x8 part: while the x shard trickles in
            # at the DMA ramp rate, each arriving MiB unlocks 3x32 matmuls
            # instead of 16, keeping the PE backlog deep enough to ride out
            # bursty arrivals (the 3 o-tiles use all 6 PSUM mains banks).
            WAVES = [[0, 1, 2], [3]] + [[i, i + 1] for i in range(4, OT, 2)]
            DEF_W = 4   # waves 0..3 = o-tiles 0..7 deferred
            N_FIX = 8
            FIX_W0 = 5  # first fixup wave
            def_tiles = {}

            def emit_fixup(ot):
                o_sb = def_tiles.pop(ot)
                for t in range(TT):
                    ts = slice(t * 512, (t + 1) * 512)
                    j = 2 * (ot % 2) + t
                    pf = psxa.tile([128, 512], F32, tag="psxa", name=f"fix{ot}_{t}")
                    nc.tensor.matmul(
                        pf[:],
                        btb2_sb[32 * j:32 * j + R, ot * 128:(ot + 1) * 128],
                        xa_sb[32 * j:32 * j + R, t * 512:(t + 1) * 512],
                        start=True,
                        stop=True,
                        tile_position=(32 * j, 0),
                    )
                    nc.vector.tensor_tensor(
                        o_sb[:, ts], o_sb[:, ts], pf[:], ADD
                    )
                    nc.gpsimd.dma_start(y_out[ot][:, ts], o_sb[:, ts])

            for wave, ots in enumerate(WAVES):
                w_tiles = []
                for ot in ots:
                    w_sb = wp.tile([128, KS, 2, 128], F8, tag="w", name=f"w{ot}")
                    nc.sync.dma_start(w_sb[:], w8_in[ot])
                    w_tiles.append(w_sb)
                pts = [
                    [
                        psp.tile([128, 512], F32, tag="pt", name=f"pt{ot}_{t}")
                        for t in range(TT)
                    ]
                    for ot in ots
                ]
                deferred = wave < DEF_W
                if wave == 0:
                    # interleave the 3 o-tiles' ks loops by x8 part
                    for c in range(XG):
                        for otl in range(len(ots)):
                            for ks in range(4 * c, 4 * c + 4):
                                for t in range(TT):
                                    nc.tensor.matmul(
                                        pts[otl][t][:],
                                        w_tiles[otl][:, ks],
                                        x8_sl(ks, t),
                                        start=(ks == 0),
                                        stop=(deferred and ks == KS - 1),
                                        perf_mode=DR,
                                    )
                else:
                    for ks in range(KS):
                        for otl in range(len(ots)):
                            for t in range(TT):
                                nc.tensor.matmul(
                                    pts[otl][t][:],
                                    w_tiles[otl][:, ks],
                                    x8_sl(ks, t),
                                    start=(ks == 0),
                                    stop=(deferred and ks == KS - 1),
                                    perf_mode=DR,
                                )
                if wave == 1:
                    emit_xbf_dmas()
                if wave == 2:
                    nc.sync.dma_start(btb_sb[:], btb_in[:])
                    nc.scalar.dma_start(btb2_sb[:], btb2_in[:])
                if wave == 4:
                    emit_xa()
                if deferred:
                    # close with a base-only epilogue; lora is patched later
                    o_sbs = [
                        dp.tile([128, T], BF16, tag="def", name=f"def{ot}")
                        for ot in ots
                    ]
                    for otl, ot in enumerate(ots):
                        def_tiles[ot] = o_sbs[otl]
                else:
                    # rank-16 lora applies, row-tiled: the 4 (ot, t) matmuls
                    # of this wave use disjoint 32-row strips of the PE array
                    # (the replicated xa/btb copies at partition 32j) so they
                    # run concurrently instead of costing 4x 512 cycles.
                    o_sbs = [
                        op.tile([128, T], BF16, tag="o", name=f"o{ot}")
                        for ot in ots
                    ]
                    for otl, ot in enumerate(ots):
                        for t in range(TT):
                            j = 2 * otl + t
                            nc.tensor.matmul(
                                pts[otl][t][:],
                                btb_sb[32 * j:32 * j + R, ot * 128:(ot + 1) * 128],
                                xa_sb[32 * j:32 * j + R, t * 512:(t + 1) * 512],
                                start=False,
                                stop=True,
                                tile_position=(32 * j, 0),
                            )
                for otl, ot in enumerate(ots):
                    for t in range(TT):
                        ts = slice(t * 512, (t + 1) * 512)
                        nc.scalar.activation(
                            o_sbs[otl][:, ts],
                            pts[otl][t][:],
                            mybir.ActivationFunctionType.Identity,
                            bias=b_sb[:, ot:ot + 1],
                            scale=1.0 / S,
                        )
                        if not deferred:
                            nc.gpsimd.dma_start(y_out[ot][:, ts], o_sbs[otl][:, ts])
                # patch one deferred o-tile per steady-state wave
                if FIX_W0 <= wave < FIX_W0 + N_FIX:
                    emit_fixup(wave - FIX_W0)

    return nc


def kernel(x, W, b, A, B):
    global LAST_RESULT
    x = np.ascontiguousarray(x, dtype=np.float32)
    W = np.ascontiguousarray(W, dtype=np.float32)

    # host layout prep (transposes so the contraction dim lands on SBUF
    # partitions; blocked so every DMA is one fully-contiguous transfer)
    xT = x.T.reshape(KC, 128, N_CORES, T).transpose(2, 1, 0, 3)  # [core, p, kc, t]
    x8_dev = np.ascontiguousarray((xT * SX)).astype(ml_dtypes.float8_e4m3)
    xbf_dev = np.ascontiguousarray(xT).astype(ml_dtypes.bfloat16)
    # [ot, p, ks, i, m]: W.T[256*ks + 128*i + p, 128*ot + m] * SW
    w8_dev = np.ascontiguousarray(
        (W.T * SW).reshape(KS, 2, 128, OT, 128).transpose(3, 2, 0, 1, 4)
    ).astype(ml_dtypes.float8_e4m3)
    at_dev = np.ascontiguousarray(
        A.T.reshape(KC, 128, R).transpose(1, 0, 2)
    ).astype(ml_dtypes.bfloat16)  # [p, kc, r]
    # [128, O]: scaled B.T replicated at partition offsets 0/32/64/96 so the
    # lora-apply matmuls can be row-tiled to 4 concurrent 32-row strips.
    # btb folds the fp8 PSUM scale S (inline applies); btb2 is unscaled (the
    # deferred fixups add into already-descaled bf16 output tiles).
    btb_dev = np.zeros((128, O), dtype=np.float32)
    btb2_dev = np.zeros((128, O), dtype=np.float32)
    for j in range(4):
        btb_dev[32 * j:32 * j + R] = S * SCALING * B.T.astype(np.float32)
        btb2_dev[32 * j:32 * j + R] = SCALING * B.T.astype(np.float32)
    b_dev = np.ascontiguousarray(
        np.asarray(b, dtype=np.float32).reshape(OT, 128).T
    )  # [p, ot]

    nc = _build_kernel()
    in_maps = [
        {
            "x8": x8_dev[c],
            "xbf": xbf_dev[c],
            "w8": w8_dev,
            "at": at_dev,
            "btb": btb_dev,
            "btb2": btb2_dev,
            "b": b_dev,
        }
        for c in range(N_CORES)
    ]
    res = run_bass_kernel_spmd(nc, in_maps, list(range(N_CORES)))
    LAST_RESULT = res

    out = np.stack(
        [res.results[c]["y"].astype(np.float32) for c in range(N_CORES)]
    )  # [c, ot, o, t]
    return np.ascontiguousarray(
        out.transpose(0, 3, 1, 2).reshape(NTOK, O)
    )
